# revision 6
# baseline (speedup 1.0000x reference)
"""Trainium2 Bass kernel for the entmax-bisect Tsallis loss (nn_BisectionLoss).

Math: the reference runs a 50-step f32 bisection per row on
f(t) = sum(relu(Xs - t)^(1/(V-1))) - 1 with Xs = 0.5*X.  The exponent
1/(V-1) is tiny, so every element strictly above t contributes ~1 and the
bisection decision at every step is [x2s > t] (x2s = second-largest Xs).
The f32 iteration provably converges to

    t_final = nextbelow(min(x2s, tmax)),   tmax = fl(m - V^(1-alpha))

(the min's second arm: rows with x2s above the bisection's upper bracket
converge to the bracket instead of x2s).  Verified bit-exact against the
50-iteration trajectory loss on the full 4096-row dataset, so the whole
75-op bisection loop collapses to 3 vector ops; nextbelow(x) for positive
normals is exactly fl(x * (1 - 2^-24)).

Device work per core (memory-bound, one pass over X):
  1. Stream X in [128, w] chunks; DVE Max8 -> per-chunk top-8, combined to
     per-row top-8.  The final row-tile uses a shrinking chunk ladder so the
     last Max8 (and everything after it) is short.
  2. Closed-form t, then sparse loss on the top-8:
     Z = relu(Xs - t)^eps (ACT ln/exp), p = Z/sum(Z),
     loss = (1 - sum(p^1.5))/0.75 + dot(p, X_top8) - X[row, target].

Rows are assigned to (tile j, partition p) as row = 4p + j so the tiny
XT/OUT transfers are 16B-contiguous per partition (128 descriptors instead
of 512).  Sharding: rows split evenly across 8 cores; no communication.
"""

from contextlib import ExitStack

import numpy as np

B, V = 4096, 32000
NCORES = 8
RB = B // NCORES  # 512 rows per core
P = 128
NT = RB // P  # 4 row-tiles per core
ALPHA = 1.5
EPS = np.float32(1.0 / (V - 1))
CVAL = np.float32(V ** (1.0 - ALPHA))
INV_DENOM = np.float32(1.0 / (ALPHA * (ALPHA - 1.0)))  # 1/0.75
NEXTBELOW = np.float32(1.0 - 2.0 ** -24)  # x*NEXTBELOW == nextbelow(x), x>0

# Chunk plan per row-tile.  Tiles stream in order.  8000-wide body chunks
# keep the DVE from building a deficit (Max8 of one chunk always finishes
# before the next lands); the last tile ends with a shrinking ladder
# (ratio >= ~0.69) so the post-stream Max8 tail is just the 1200-wide chunk.
PLAN = [
    [8000, 8000, 8000, 8000],
    [8000, 8000, 8000, 8000],
    [8000, 8000, 8000, 8000],
    [8000, 8000, 6000, 4100, 2800, 1900, 1200],
]
assert all(sum(p) == V for p in PLAN) and len(PLAN) == NT

_CACHE: dict = {}


def _build():
    import concourse.bass as bass  # noqa: F401
    import concourse.tile as tile
    from concourse import bacc, mybir

    f32 = mybir.dt.float32
    AX = mybir.AxisListType.X
    Alu = mybir.AluOpType
    Act = mybir.ActivationFunctionType

    nc = bacc.Bacc(
        "TRN2", target_bir_lowering=False, debug=False, enable_asserts=False
    )
    Xp = nc.declare_dram_parameter("X", [RB, V], f32, isOutput=False)
    XTp = nc.declare_dram_parameter("XT", [RB], f32, isOutput=False)
    OUTp = nc.declare_dram_parameter("OUT", [RB], f32, isOutput=True)
    # row (4p + j) <-> (tile j, partition p)
    X3 = Xp.ap().rearrange("(p j) v -> j p v", j=NT)
    XT2 = XTp.ap().rearrange("(p j) -> p j", j=NT)
    OUT2 = OUTp.ap().rearrange("(p j) -> p j", j=NT)

    with tile.TileContext(nc) as tc, ExitStack() as ctx:
        xpool = ctx.enter_context(tc.tile_pool(name="xc", bufs=6))
        sp = ctx.enter_context(tc.tile_pool(name="small", bufs=1))

        nch = sum(len(p) for p in PLAN)
        cand = sp.tile([P, nch * 8], f32)
        top8 = sp.tile([P, NT * 8], f32)
        xt = sp.tile([P, NT], f32)
        lossT = sp.tile([P, NT], f32)

        cseq = [0]  # global chunk counter (ring alternation + cand slot)

        def stream_tile(j):
            k0 = cseq[0]
            col = 0
            for w in PLAN[j]:
                xt_ = xpool.tile([P, w], f32, tag="xc")
                eng = nc.scalar if cseq[0] % 2 else nc.sync
                eng.dma_start(xt_[:], X3[j, :, col : col + w])
                k = cseq[0] * 8
                nc.vector.max(cand[:, k : k + 8], xt_[:])
                cseq[0] += 1
                col += w
            nc.vector.max(
                top8[:, j * 8 : (j + 1) * 8],
                cand[:, k0 * 8 : cseq[0] * 8],
            )

        def loss_range(jlo, jhi):
            """Closed-form t + sparse loss on the top-8 for tiles [jlo, jhi)."""
            n = jhi - jlo
            t8 = top8[:, jlo * 8 : jhi * 8]    # [P, n*8] raw X top-8, desc
            x1 = t8[:, 0 : n * 8 : 8]          # [P, n]
            x2 = t8[:, 1 : n * 8 : 8]
            v3 = t8.rearrange("p (j k) -> p j k", k=8)  # [P, n, 8]

            tmax = sp.tile([P, n], f32, tag=f"tm{jlo}")
            t = sp.tile([P, n], f32, tag=f"t{jlo}")
            nc.vector.tensor_scalar(
                out=tmax[:], in0=x1, scalar1=0.5, scalar2=float(CVAL),
                op0=Alu.mult, op1=Alu.subtract,
            )
            nc.vector.scalar_tensor_tensor(
                out=t[:], in0=x2, scalar=0.5, in1=tmax[:],
                op0=Alu.mult, op1=Alu.min,
            )  # ub = min(0.5*x2, tmax)
            nc.vector.tensor_scalar_mul(t[:], t[:], float(NEXTBELOW))

            tb = t[:].rearrange("p (j one) -> p j one", one=1).broadcast_to([P, n, 8])
            u = sp.tile([P, n, 8], f32, tag=f"u{jlo}")
            nc.vector.scalar_tensor_tensor(
                out=u[:], in0=v3, scalar=0.5, in1=tb,
                op0=Alu.mult, op1=Alu.subtract,
            )  # u = Xs - t
            msk = sp.tile([P, n, 8], f32, tag=f"mk{jlo}")
            nc.vector.tensor_scalar(
                out=msk[:], in0=u[:], scalar1=0.0, scalar2=None, op0=Alu.is_gt
            )
            nc.vector.tensor_scalar_max(u[:], u[:], 1e-38)
            nc.scalar.activation(u[:], u[:], Act.Ln)
            nc.scalar.activation(u[:], u[:], Act.Exp, scale=float(EPS))
            Z = sp.tile([P, n, 8], f32, tag=f"z{jlo}")
            nc.vector.tensor_mul(Z[:], u[:], msk[:])
            S1 = sp.tile([P, n], f32, tag=f"s1{jlo}")
            nc.vector.reduce_sum(
                S1[:].rearrange("p (j one) -> p j one", one=1), Z[:], axis=AX
            )
            rcp = sp.tile([P, n], f32, tag=f"rc{jlo}")
            nc.vector.reciprocal(rcp[:], S1[:])
            rb = rcp[:].rearrange("p (j one) -> p j one", one=1).broadcast_to([P, n, 8])
            p = sp.tile([P, n, 8], f32, tag=f"p{jlo}")
            nc.vector.scalar_tensor_tensor(
                out=p[:], in0=Z[:], scalar=1.0, in1=rb, op0=Alu.mult, op1=Alu.mult
            )
            # p^1.5 via exp(1.5*ln(p)) keeps the tail on the already-resident
            # Ln/Exp tables (Sqrt lives in a third table -> 1.3us swap).
            # Masked lanes: p=0 -> clamp 1e-38 -> exp(1.5*ln) ~ 4e-57 -> 0.
            sq = sp.tile([P, n, 8], f32, tag=f"sq{jlo}")
            nc.vector.tensor_scalar_max(sq[:], p[:], 1e-38)
            nc.scalar.activation(sq[:], sq[:], Act.Ln)
            nc.scalar.activation(sq[:], sq[:], Act.Exp, scale=1.5)
            Sa = sp.tile([P, n], f32, tag=f"sa{jlo}")
            nc.vector.reduce_sum(
                Sa[:].rearrange("p (j one) -> p j one", one=1), sq[:], axis=AX
            )
            q = sp.tile([P, n], f32, tag=f"q{jlo}")
            nc.vector.tensor_scalar(
                out=q[:], in0=Sa[:], scalar1=1.0, scalar2=float(INV_DENOM),
                op0=Alu.subtract, op1=Alu.mult,
            )  # (Sa-1)/0.75 == -(1-Sa)/0.75
            nc.vector.tensor_mul(p[:], p[:], v3)  # p * X_top8
            D = sp.tile([P, n], f32, tag=f"d{jlo}")
            nc.vector.reduce_sum(
                D[:].rearrange("p (j one) -> p j one", one=1), p[:], axis=AX
            )
            nc.vector.scalar_tensor_tensor(
                out=D[:], in0=q[:], scalar=-1.0, in1=D[:],
                op0=Alu.mult, op1=Alu.add,
            )  # D - q
            nc.vector.tensor_sub(lossT[:, jlo:jhi], D[:], xt[:, jlo:jhi])

        stream_tile(0)
        # Tiny strided gather rides the scalar ring behind tile 0's chunks.
        nc.scalar.dma_start(xt[:], XT2)
        # Warm the Ln/Exp/Sqrt activation tables now so their ACT_TABLE_LOADs
        # hide under the stream instead of landing in the kernel tail.
        warm = sp.tile([P, 8], f32)
        nc.gpsimd.memset(warm[:], 1.0)
        nc.scalar.activation(warm[:], warm[:], Act.Ln)
        nc.scalar.activation(warm[:], warm[:], Act.Exp, scale=float(EPS))
        for j in range(1, NT):
            stream_tile(j)
            if j == NT - 2:
                loss_range(0, NT - 1)  # tiles 0..2 hidden under tile 3's stream
        loss_range(NT - 1, NT)

        nc.sync.dma_start(OUT2, lossT[:])

    nc.compile()
    return nc


def get_nc():
    if "nc" not in _CACHE:
        _CACHE["nc"] = _build()
    return _CACHE["nc"]


def kernel(X: np.ndarray, target: np.ndarray) -> np.ndarray:
    from concourse.bass_utils import run_bass_kernel_spmd

    X = np.ascontiguousarray(np.asarray(X, dtype=np.float32))
    target = np.asarray(target)
    assert X.shape == (B, V) and target.shape == (B,)

    xt = X[np.arange(B), target.astype(np.int64)].astype(np.float32)

    nc = get_nc()
    in_maps = [
        {
            "X": X[c * RB : (c + 1) * RB],
            "XT": xt[c * RB : (c + 1) * RB],
        }
        for c in range(NCORES)
    ]
    res = run_bass_kernel_spmd(nc, in_maps, core_ids=list(range(NCORES))).results
    return np.concatenate([res[c]["OUT"] for c in range(NCORES)], axis=0)


# revision 9
# speedup vs baseline: 1.2654x; 1.2654x over previous
"""Trainium2 Bass kernel for the entmax-bisect Tsallis loss (nn_BisectionLoss).

Math: the reference runs a 50-step f32 bisection per row on
f(t) = sum(relu(Xs - t)^(1/(V-1))) - 1 with Xs = 0.5*X.  The exponent
1/(V-1) is tiny, so every element strictly above t contributes ~1 and the
bisection decision at every step is [x2s > t] (x2s = second-largest Xs).
The f32 iteration provably converges to

    t_final = nextbelow(min(x2s, tmax)),   tmax = fl(m - V^(1-alpha))

(the min's second arm: rows with x2s above the bisection's upper bracket
converge to the bracket instead of x2s).  Verified bit-exact against the
50-iteration trajectory loss on the full 4096-row dataset, so the whole
75-op bisection loop collapses to 3 vector ops; nextbelow(x) for positive
normals is exactly fl(x * (1 - 2^-24)).

Device work per core (memory-bound, one pass over X):
  1. Stream X in [128, w] chunks; DVE Max8 -> per-chunk top-8, combined to
     per-row top-8.  The final row-tile uses a shrinking chunk ladder so the
     last Max8 (and everything after it) is short.
  2. Closed-form t, then sparse loss on the top-8:
     Z = relu(Xs - t)^eps (ACT ln/exp), p = Z/sum(Z),
     loss = (1 - sum(p^1.5))/0.75 + dot(p, X_top8) - X[row, target].

Rows are assigned to (tile j, partition p) as row = 4p + j so the tiny
XT/OUT transfers are 16B-contiguous per partition (128 descriptors instead
of 512).  Sharding: rows split evenly across 8 cores; no communication.
"""

from contextlib import ExitStack

import numpy as np

B, V = 4096, 32000
NCORES = 8
RB = B // NCORES  # 512 rows per core
P = 128
NT = RB // P  # 4 row-tiles per core
ALPHA = 1.5
EPS = np.float32(1.0 / (V - 1))
CVAL = np.float32(V ** (1.0 - ALPHA))
INV_DENOM = np.float32(1.0 / (ALPHA * (ALPHA - 1.0)))  # 1/0.75
NEXTBELOW = np.float32(1.0 - 2.0 ** -24)  # x*NEXTBELOW == nextbelow(x), x>0
# Float-bits log: for u>0, log2(u) ~ (bits_i32(u) - 127*2^23)/2^23 (max err
# 0.086).  Z = u^eps = exp(eps*ln u) = 1 + eps*ln(u) + O(6e-8), and eps*0.086
# *ln2 = 1.9e-6, so Z computed as (bits - BIAS)*BSCL + 1 matches the exact
# table-based Z to ~2e-6 -- which the loss amplifies to <5e-7 relative
# (validated on the full dataset).  Kills every ACT Ln/Exp and table load.
BIAS = np.float32(127 * 2 ** 23)
BSCL = np.float32(float(EPS) * np.log(2.0) / 2 ** 23)

# Chunk plan per row-tile.  Tiles stream in order.  8000-wide body chunks
# keep the DVE from building a deficit (Max8 of one chunk always finishes
# before the next lands); the last tile ends with a shrinking ladder
# (ratio >= ~0.69) so the post-stream Max8 tail is just the 1200-wide chunk.
PLAN = [
    [8000, 8000, 8000, 8000],
    [8000, 8000, 8000, 8000],
    [8000, 8000, 8000, 8000],
    [8000, 8000, 6000, 4100, 2800, 1900, 1200],
]
assert all(sum(p) == V for p in PLAN) and len(PLAN) == NT

_CACHE: dict = {}


def _build():
    import concourse.bass as bass  # noqa: F401
    import concourse.tile as tile
    from concourse import bacc, mybir

    f32 = mybir.dt.float32
    AX = mybir.AxisListType.X
    Alu = mybir.AluOpType
    Act = mybir.ActivationFunctionType

    nc = bacc.Bacc(
        "TRN2", target_bir_lowering=False, debug=False, enable_asserts=False
    )
    Xp = nc.declare_dram_parameter("X", [RB, V], f32, isOutput=False)
    XTp = nc.declare_dram_parameter("XT", [RB], f32, isOutput=False)
    OUTp = nc.declare_dram_parameter("OUT", [RB], f32, isOutput=True)
    # row (4p + j) <-> (tile j, partition p)
    X3 = Xp.ap().rearrange("(p j) v -> j p v", j=NT)
    XT2 = XTp.ap().rearrange("(p j) -> p j", j=NT)
    OUT2 = OUTp.ap().rearrange("(p j) -> p j", j=NT)

    with tile.TileContext(nc) as tc, ExitStack() as ctx:
        xpool = ctx.enter_context(tc.tile_pool(name="xc", bufs=6))
        sp = ctx.enter_context(tc.tile_pool(name="small", bufs=1))

        nch = sum(len(p) for p in PLAN)
        cand = sp.tile([P, nch * 8], f32)
        top8 = sp.tile([P, NT * 8], f32)
        xt = sp.tile([P, NT], f32)
        lossT = sp.tile([P, NT], f32)

        cseq = [0]  # global chunk counter (ring alternation + cand slot)

        def stream_tile(j):
            k0 = cseq[0]
            col = 0
            for w in PLAN[j]:
                xt_ = xpool.tile([P, w], f32, tag="xc")
                eng = nc.scalar if cseq[0] % 2 else nc.sync
                eng.dma_start(xt_[:], X3[j, :, col : col + w])
                k = cseq[0] * 8
                nc.vector.max(cand[:, k : k + 8], xt_[:])
                cseq[0] += 1
                col += w
            nc.vector.max(
                top8[:, j * 8 : (j + 1) * 8],
                cand[:, k0 * 8 : cseq[0] * 8],
            )

        def loss_range(jlo, jhi):
            """Closed-form t + sparse loss on the top-8 for tiles [jlo, jhi)."""
            n = jhi - jlo
            t8 = top8[:, jlo * 8 : jhi * 8]    # [P, n*8] raw X top-8, desc
            x1 = t8[:, 0 : n * 8 : 8]          # [P, n]
            x2 = t8[:, 1 : n * 8 : 8]
            v3 = t8.rearrange("p (j k) -> p j k", k=8)  # [P, n, 8]

            tmax = sp.tile([P, n], f32, tag=f"tm{jlo}")
            t = sp.tile([P, n], f32, tag=f"t{jlo}")
            nc.vector.tensor_scalar(
                out=tmax[:], in0=x1, scalar1=0.5, scalar2=float(CVAL),
                op0=Alu.mult, op1=Alu.subtract,
            )
            nc.vector.scalar_tensor_tensor(
                out=t[:], in0=x2, scalar=0.5, in1=tmax[:],
                op0=Alu.mult, op1=Alu.min,
            )  # ub = min(0.5*x2, tmax)
            nc.vector.tensor_scalar_mul(t[:], t[:], float(NEXTBELOW))

            tb = t[:].rearrange("p (j one) -> p j one", one=1).broadcast_to([P, n, 8])
            u = sp.tile([P, n, 8], f32, tag=f"u{jlo}")
            nc.vector.scalar_tensor_tensor(
                out=u[:], in0=v3, scalar=0.5, in1=tb,
                op0=Alu.mult, op1=Alu.subtract,
            )  # u = Xs - t
            msk = sp.tile([P, n, 8], f32, tag=f"mk{jlo}")
            nc.vector.tensor_scalar(
                out=msk[:], in0=u[:], scalar1=0.0, scalar2=None, op0=Alu.is_gt
            )
            # Z = u^eps via float-bits log (see BIAS/BSCL above); u<=0 lanes
            # produce garbage y but msk zeroes them in the same fused op.
            yp = sp.tile([P, n, 8], f32, tag=f"yp{jlo}")
            nc.vector.tensor_scalar(
                out=yp[:], in0=u[:].bitcast(mybir.dt.int32), scalar1=float(BIAS),
                scalar2=float(BSCL), op0=Alu.subtract, op1=Alu.mult,
            )
            Z = sp.tile([P, n, 8], f32, tag=f"z{jlo}")
            nc.vector.scalar_tensor_tensor(
                out=Z[:], in0=yp[:], scalar=1.0, in1=msk[:],
                op0=Alu.add, op1=Alu.mult,
            )  # Z = (1 + eps*ln(u)) * msk
            S1 = sp.tile([P, n], f32, tag=f"s1{jlo}")
            nc.vector.reduce_sum(
                S1[:].rearrange("p (j one) -> p j one", one=1), Z[:], axis=AX
            )
            kk = sp.tile([P, n], f32, tag=f"kk{jlo}")
            nc.vector.reduce_sum(
                kk[:].rearrange("p (j one) -> p j one", one=1), msk[:], axis=AX
            )
            rcp = sp.tile([P, n], f32, tag=f"rc{jlo}")
            nc.vector.reciprocal(rcp[:], S1[:])
            rk = sp.tile([P, n], f32, tag=f"rk{jlo}")
            nc.vector.reciprocal(rk[:], kk[:])
            rb = rcp[:].rearrange("p (j one) -> p j one", one=1).broadcast_to([P, n, 8])
            p = sp.tile([P, n, 8], f32, tag=f"p{jlo}")
            nc.vector.scalar_tensor_tensor(
                out=p[:], in0=Z[:], scalar=1.0, in1=rb, op0=Alu.mult, op1=Alu.mult
            )
            # Sa = sum p^1.5 = k^-0.5 * (1 + 0.375*var(eps*ln u)) -- the
            # first-order tilt cancels exactly; the var term is < 1e-7.
            Sa = sp.tile([P, n], f32, tag=f"sa{jlo}")
            nc.scalar.activation(Sa[:], rk[:], Act.Sqrt)
            q = sp.tile([P, n], f32, tag=f"q{jlo}")
            nc.vector.tensor_scalar(
                out=q[:], in0=Sa[:], scalar1=1.0, scalar2=float(INV_DENOM),
                op0=Alu.subtract, op1=Alu.mult,
            )  # (Sa-1)/0.75 == -(1-Sa)/0.75
            nc.vector.tensor_mul(p[:], p[:], v3)  # p * X_top8
            D = sp.tile([P, n], f32, tag=f"d{jlo}")
            nc.vector.reduce_sum(
                D[:].rearrange("p (j one) -> p j one", one=1), p[:], axis=AX
            )
            nc.vector.scalar_tensor_tensor(
                out=D[:], in0=q[:], scalar=-1.0, in1=D[:],
                op0=Alu.mult, op1=Alu.add,
            )  # D - q
            nc.vector.tensor_sub(lossT[:, jlo:jhi], D[:], xt[:, jlo:jhi])

        stream_tile(0)
        # Tiny strided gather rides the scalar ring behind tile 0's chunks.
        nc.scalar.dma_start(xt[:], XT2)
        # Warm the Sqrt activation table (the only ACT function used) so its
        # ACT_TABLE_LOAD hides under the stream instead of the kernel tail.
        warm = sp.tile([P, 8], f32)
        nc.gpsimd.memset(warm[:], 1.0)
        nc.scalar.activation(warm[:], warm[:], Act.Sqrt)
        for j in range(1, NT):
            stream_tile(j)
            if j == NT - 2:
                loss_range(0, NT - 1)  # tiles 0..2 hidden under tile 3's stream
        loss_range(NT - 1, NT)

        nc.sync.dma_start(OUT2, lossT[:])

    nc.compile()
    return nc


def get_nc():
    if "nc" not in _CACHE:
        _CACHE["nc"] = _build()
    return _CACHE["nc"]


def kernel(X: np.ndarray, target: np.ndarray) -> np.ndarray:
    from concourse.bass_utils import run_bass_kernel_spmd

    X = np.ascontiguousarray(np.asarray(X, dtype=np.float32))
    target = np.asarray(target)
    assert X.shape == (B, V) and target.shape == (B,)

    xt = X[np.arange(B), target.astype(np.int64)].astype(np.float32)

    nc = get_nc()
    in_maps = [
        {
            "X": X[c * RB : (c + 1) * RB],
            "XT": xt[c * RB : (c + 1) * RB],
        }
        for c in range(NCORES)
    ]
    res = run_bass_kernel_spmd(nc, in_maps, core_ids=list(range(NCORES))).results
    return np.concatenate([res[c]["OUT"] for c in range(NCORES)], axis=0)
